# revision 7
# baseline (speedup 1.0000x reference)
"""Multi-Head Latent Attention (MLA) prefill kernel for 8 Trainium2 NeuronCores.

v2: bf16 datapath (PE at full 2.4GHz clock vs fp32r's 1.2GHz), split
AllGather (q | kv+rope) and AllToAll (per head) for comm/compute overlap,
softmax denominators accumulated on the Vector engine instead of PE
ones-matmuls, wo preloaded during attention.

Sharding: latent down-projections row-split 8 ways + AllGather; up-projections
and attention head-split (2 heads/core); AllToAll converts head-split attention
output to token-split for the output projection.
"""
import sys
if '/opt/trn_rl_repo' not in sys.path:
    sys.path.insert(0, '/opt/trn_rl_repo')

import math
import numpy as np
import ml_dtypes

import concourse.bass as bass
import concourse.tile as tile
import concourse.mybir as mybir
from concourse import bacc

F32 = mybir.dt.float32
F32R = mybir.dt.float32r
BF16 = mybir.dt.bfloat16
F8 = mybir.dt.float8e4
AF = mybir.ActivationFunctionType
ALU = mybir.AluOpType
PM = mybir.MatmulPerfMode
NPBF = ml_dtypes.bfloat16
WS = 16.0              # q/k pre-scale: centers fp8e4m3 away from subnormals

B, S, DIM, H = 2, 2048, 2048, 16
NOPE, ROPE, QKD, VD = 128, 64, 192, 128
QLR, KVLR = 512, 512
EPS = 1e-6
NC = 8
N = B * S              # 4096 flattened tokens
R = N // NC            # 512 tokens per core (phase 1 / output rows)
HPC = H // NC          # 2 heads per core
NBLK = N // R          # 8 token blocks (= AG shards)
SCALE = 1.0 / math.sqrt(QKD)

SKIP, PLAIN = -2, -1   # mask block classes (>=0 -> index into mask blocks)


def _rope_tables():
    freqs = (1.0 / (10000.0 ** (np.arange(0, ROPE, 2, dtype=np.float32) / ROPE)))
    ang = np.arange(S, dtype=np.float32)[:, None] * freqs[None, :]      # [S, 32]
    return np.cos(ang).T.copy(), np.sin(ang).T.copy()                   # [32, S]


def _classify_mask(mask):
    """Per (q-chunk of 512, k-block of 128): SKIP / PLAIN / index of mask block.

    Returns (cls[4][16], blocks [nblk,128,512] transposed mask)."""
    cls = [[PLAIN] * (S // 128) for _ in range(S // 512)]
    blocks = []
    for qc in range(S // 512):
        sub_q = mask[qc * 512:(qc + 1) * 512]
        for kb in range(S // 128):
            blk = sub_q[:, kb * 128:(kb + 1) * 128]
            if not blk.any():
                cls[qc][kb] = SKIP
            elif blk.all():
                cls[qc][kb] = PLAIN
            else:
                cls[qc][kb] = len(blocks)
                blocks.append(blk.T.astype(NPBF))   # [128 k, 512 q]
    blocks = (np.stack(blocks) if blocks
              else np.zeros((0, 128, 512), NPBF))
    return cls, blocks


def _build(cls, nmask, flags, repeat=1):
    """Emit the bass program. cls/nmask/flags are compile-time schedule data."""
    nc = bacc.Bacc(None, num_devices=NC)

    # ---- I/O ----
    x_c = nc.dram_tensor("x_c", [DIM // 128, 128, R], BF16, kind="ExternalInput")
    # phase-1 weights m-major along columns: one wide DMA per k-chunk
    wqaT = nc.dram_tensor("wqaT", [DIM // 128, 128, 4 * 128], BF16, kind="ExternalInput")
    bqa = nc.dram_tensor("bqa", [QLR], F32, kind="ExternalInput")
    wkvaT = nc.dram_tensor("wkvaT", [DIM // 128, 128, 5 * 128], BF16, kind="ExternalInput")
    bkva = nc.dram_tensor("bkva", [KVLR + ROPE], F32, kind="ExternalInput")
    qnw = nc.dram_tensor("qnw", [QLR], F32, kind="ExternalInput")
    kvnw = nc.dram_tensor("kvnw", [KVLR], F32, kind="ExternalInput")
    # trig: phase-1 (this core's 512 positions) & phase-2 (full 2048 positions)
    trig1c = nc.dram_tensor("trig1c", [64, R], F32, kind="ExternalInput")   # [cos;sin]
    trig2c = nc.dram_tensor("trig2c", [64, R], F32, kind="ExternalInput")   # [sin;cos]
    trigq1 = nc.dram_tensor("trigq1", [128, S], F32, kind="ExternalInput")  # [c;s;c;s]
    trigq2 = nc.dram_tensor("trigq2", [128, S], F32, kind="ExternalInput")  # [s;c;s;c]
    sgn = nc.dram_tensor("sgn", [128, 1], F32, kind="ExternalInput")        # rope comb sign
    # m: 0/1 = nope h0/h1, 2 = rope rows, 3 = rope rows with e/o groups swapped
    wqbT = nc.dram_tensor("wqbT", [4, 4, 128, 128], BF16, kind="ExternalInput")
    bqb = nc.dram_tensor("bqb", [4 * 128], F32, kind="ExternalInput")
    wkbT = nc.dram_tensor("wkbT", [4, 2, 128, 128], BF16, kind="ExternalInput")
    bkb = nc.dram_tensor("bkb", [HPC * NOPE], F32, kind="ExternalInput")
    wvbT = nc.dram_tensor("wvbT", [4, 128, HPC * VD], BF16, kind="ExternalInput")
    bvb = nc.dram_tensor("bvb", [1, HPC * VD], F32, kind="ExternalInput")
    woT = nc.dram_tensor("woT", [16, 4, 128, 512], BF16, kind="ExternalInput")
    wob = nc.dram_tensor("wob", [1, DIM], F32, kind="ExternalInput")
    maskblk = nc.dram_tensor("maskblk", [max(nmask, 1), 128, 512], BF16,
                             kind="ExternalInput")
    out_c = nc.dram_tensor("out", [R, DIM], F32, kind="ExternalOutput")

    with tile.TileContext(nc) as tc:
        with tc.tile_pool(name="konst", bufs=1) as konst, \
             tc.tile_pool(name="dram", bufs=1, space="DRAM") as dram:

            # ---- constants ----
            ones_f = konst.tile([128, 1], F32)
            nc.vector.memset(ones_f[:, :], 1.0)
            ones_bf = konst.tile([128, 1], BF16)
            nc.vector.memset(ones_bf[:, :], 1.0)
            ones_sq = konst.tile([128, 128], BF16)
            nc.vector.memset(ones_sq[:, :], 1.0)
            ones_rf = konst.tile([1, 128], F32)
            nc.vector.memset(ones_rf[:, :], 1.0)
            ones_row = konst.tile([1, 128], F32R)
            nc.vector.tensor_copy(out=ones_row[:, :], in_=ones_rf[:, :])
            sgn_t = konst.tile([128, 1], F32)
            nc.sync.dma_start(out=sgn_t, in_=sgn[:, :])
            eps_t = konst.tile([1, 1], F32)
            nc.vector.memset(eps_t[:, :], EPS)

            for _rep in range(repeat):
                qkv = tc.alloc_tile_pool(name=f"qkv{_rep}", bufs=1)
                # ---- collective DRAM tiles ----
                agq_in = dram.tile([QLR, R], BF16, name=f"agqi{_rep}")
                agq_out = dram.tile([NC, QLR, R], BF16, addr_space="Shared",
                                    name=f"agqo{_rep}")
                agk_in = dram.tile([KVLR + ROPE, R], BF16, name=f"agki{_rep}")
                agk_out = dram.tile([NC, KVLR, R], BF16, addr_space="Shared",
                                    name=f"agko{_rep}")
                agr_out = dram.tile([NC, ROPE, R], BF16, addr_space="Shared",
                                    name=f"agro{_rep}")
                a2a_in = [dram.tile([NC, VD, R], BF16, name=f"a2ai{lh}_{_rep}")
                          for lh in range(HPC)]
                a2a_out = [dram.tile([NC, VD, R], BF16, name=f"a2ao{lh}_{_rep}")
                           for lh in range(HPC)]
                # ---- persistent q/k/v (phase 2 -> phase 3) ----
                # q/k in fp8 DR layout [contract, slot, token]: slot0 = nope,
                # slot1 = rope (rows 0-63) + zero pad; values are x WS.
                qdr = [qkv.tile([128, 2, N], F8, name=f"qdr{i}") for i in range(HPC)]
                kdr = [qkv.tile([128, 2, N], F8, name=f"kdr{i}") for i in range(HPC)]
                vt = qkv.tile([128, N // 128, HPC * VD], BF16)   # token-major V
                for i in range(HPC):
                    nc.vector.memset(qdr[i][64:128, 1, :], 0.0)
                    nc.vector.memset(kdr[i][64:128, 1, :], 0.0)

                # ================= PHASE 1: latent down-proj (row shard) ============
                with tc.tile_pool(name=f"p1sb{_rep}", bufs=1) as p1sb, \
                     tc.tile_pool(name=f"p1x{_rep}", bufs=1) as p1x, \
                     tc.tile_pool(name=f"p1w{_rep}", bufs=8) as p1w, \
                     tc.tile_pool(name=f"p1tmp{_rep}", bufs=1) as p1tmp, \
                     tc.tile_pool(name=f"p1ps{_rep}", bufs=1, space="PSUM") as p1ps, \
                     tc.tile_pool(name=f"p1ps2{_rep}", bufs=2, space="PSUM") as p1ps2, \
                     tc.tile_pool(name=f"p1ps1{_rep}", bufs=1, space="PSUM") as p1ps1:

                    t1c = p1sb.tile([64, R], F32)
                    nc.sync.dma_start(out=t1c, in_=trig1c[:, :])
                    t2c = p1sb.tile([64, R], F32)
                    nc.sync.dma_start(out=t2c, in_=trig2c[:, :])

                    xts = [p1x.tile([128, R], BF16, tag=f"xt{k}", name=f"xt{k}")
                           for k in range(DIM // 128)]
                    xt_loaded = [False] * (DIM // 128)

                    # kv path FIRST so AG-kv overlaps the q path, and AG-q
                    # overlaps phase-2's k/v pass.
                    for path in (1, 0):
                        wT, bias_d, normw_d = ((wqaT, bqa, qnw) if path == 0
                                               else (wkvaT, bkva, kvnw))
                        nm = 4 if path == 0 else 5  # kv has extra 64-row rope chunk
                        sums_ps = p1ps1.tile([1, R], F32, name=f"sums{path}",
                                             tag="sums")
                        # k-outer: 4-5 live PSUM accumulators, x tiles stream in
                        pss = [p1ps.tile([128, R], F32, tag=f"p1acc{m}",
                                         name=f"ps{path}{m}") for m in range(nm)]
                        for k in range(DIM // 128):
                            if not xt_loaded[k]:
                                nc.sync.dma_start(out=xts[k], in_=x_c[k, :, :])
                                xt_loaded[k] = True
                            wt = p1w.tile([128, nm * 128], BF16, tag=f"w1p{path}")
                            nc.sync.dma_start(out=wt[:, :], in_=wT[k, :, :])
                            for m in range(nm):
                                nc.tensor.matmul(pss[m][:, :],
                                                 wt[:, m * 128:(m + 1) * 128],
                                                 xts[k][:, :],
                                                 start=(k == 0), stop=(k == DIM // 128 - 1),
                                                 skip_group_check=True)
                        acts = []
                        for m in range(nm):
                            mp = 64 if m == 4 else 128
                            ps = pss[m]
                            a = p1sb.tile([128, R], F32, tag=f"act{m}",
                                          name=f"a{path}{m}")
                            if flags['ba'][path]:
                                bt = p1sb.tile([128, 1], F32, tag=f"bias{m}",
                                               name=f"b{path}{m}")
                                nc.sync.dma_start(out=bt[0:mp, :],
                                                  in_=bias_d[m * 128:m * 128 + mp]
                                                  .rearrange("(a b) -> a b", b=1))
                                nc.vector.tensor_scalar_add(a[0:mp, :], ps[0:mp, :],
                                                            bt[0:mp, :])
                            else:
                                nc.scalar.activation(out=a[0:mp, :], in_=ps[0:mp, :],
                                                     func=AF.Copy)
                            acts.append(a)
                            if m < 4:   # latent chunks: accumulate sum of squares
                                sq = p1tmp.tile([128, R], BF16, tag="sq")
                                nc.vector.tensor_mul(sq[:, :], a[:, :], a[:, :])
                                nc.tensor.matmul(sums_ps[:, :], ones_bf[:, :], sq[:, :],
                                                 start=(m == 0), stop=(m == 3),
                                                 skip_group_check=True)
                        # rstd = 1/sqrt(mean + eps)
                        std = p1tmp.tile([1, R], F32, tag="std")
                        nc.scalar.activation(out=std[:, :], in_=sums_ps[:, :],
                                             func=AF.Sqrt,
                                             scale=1.0 / (QLR if path == 0 else KVLR),
                                             bias=eps_t[:, :])
                        rstd_f = p1tmp.tile([1, R], F32, tag="rstdf")
                        nc.vector.reciprocal(out=rstd_f[:, :], in_=std[:, :])
                        rstd = p1tmp.tile([1, R], F32R, tag="rstd")
                        nc.vector.tensor_copy(out=rstd[:, :], in_=rstd_f[:, :])
                        ag_dst = agq_in if path == 0 else agk_in
                        for m in range(4):
                            wrow = p1sb.tile([1, 128], F32R, tag=f"wrow{m}",
                                             name=f"w{path}{m}")
                            nc.sync.dma_start(out=wrow,
                                              in_=normw_d[m * 128:(m + 1) * 128]
                                              .rearrange("(b a) -> b a", b=1).bitcast(F32R))
                            rep = p1ps2.tile([128, R], F32, tag="p1rep")
                            nc.tensor.matmul(rep[:, :], wrow[:, :], rstd[:, :],
                                             start=True, stop=True)
                            nrm = p1tmp.tile([128, R], BF16, tag="nrm")
                            nc.vector.tensor_mul(nrm[:, :], acts[m][:, :], rep[:, :])
                            nc.sync.dma_start(
                                out=ag_dst[m * 128:(m + 1) * 128, :],
                                in_=nrm[:, :])
                        if path == 1:
                            # latent shard ready -> big AG piece flies while the
                            # rope chain and the q path still compute
                            nc.gpsimd.collective_compute(
                                "AllGather", ALU.bypass,
                                replica_groups=[list(range(NC))],
                                ins=[agk_in[0:KVLR, :].opt()], outs=[agk_out.opt()])
                        if path == 1:   # rope on k_pe chunk [64, R]
                            kpe = acts[4]
                            u = p1tmp.tile([64, R], F32, tag="u1")
                            nc.vector.tensor_mul(u[:, :], kpe[0:64, :], t1c[:, :])
                            w = p1tmp.tile([64, R], F32, tag="w1t")
                            nc.vector.tensor_mul(w[:, :], kpe[0:64, :], t2c[:, :])
                            z1 = p1tmp.tile([64, R], F32, tag="z1")
                            nc.sync.dma_start(out=z1[0:32, :], in_=u[32:64, :])
                            nc.sync.dma_start(out=z1[32:64, :], in_=w[0:32, :])
                            y1 = p1tmp.tile([64, R], F32, tag="y1")
                            nc.sync.dma_start(out=y1[0:32, :], in_=u[0:32, :])
                            nc.sync.dma_start(out=y1[32:64, :], in_=w[32:64, :])
                            rot = p1tmp.tile([64, R], BF16, tag="rot1")
                            nc.vector.scalar_tensor_tensor(
                                out=rot[:, :], in0=z1[:, :], scalar=sgn_t[0:64, :],
                                in1=y1[:, :], op0=ALU.mult, op1=ALU.add)
                            nc.sync.dma_start(out=agk_in[KVLR:KVLR + ROPE, :],
                                              in_=rot[:, :])
                            # small rope piece
                            nc.gpsimd.collective_compute(
                                "AllGather", ALU.bypass,
                                replica_groups=[list(range(NC))],
                                ins=[agk_in[KVLR:KVLR + ROPE, :].opt()],
                                outs=[agr_out.opt()])
                    # AG-q overlaps phase-2's k/v pass
                    nc.gpsimd.collective_compute(
                        "AllGather", ALU.bypass,
                        replica_groups=[list(range(NC))],
                        ins=[agq_in.opt()], outs=[agq_out.opt()])

                # ================= PHASE 2: per-head up-projections ================
                with tc.tile_pool(name=f"p2w{_rep}", bufs=1) as p2w, \
                     tc.tile_pool(name=f"p2lat{_rep}", bufs=2) as p2lat, \
                     tc.tile_pool(name=f"p2tmp{_rep}", bufs=2) as p2tmp, \
                     tc.tile_pool(name=f"p2ps{_rep}", bufs=2, space="PSUM") as p2ps:

                    tq1 = p2w.tile([128, S], F32)
                    nc.sync.dma_start(out=tq1, in_=trigq1[:, :])
                    tq2 = p2w.tile([128, S], F32)
                    nc.sync.dma_start(out=tq2, in_=trigq2[:, :])
                    # weight tiles (held)
                    wqb_t = [[p2w.tile([128, 128], BF16, name=f"wqb{k}{m}")
                              for m in range(4)] for k in range(4)]
                    for k in range(4):
                        for m in range(4):
                            nc.sync.dma_start(out=wqb_t[k][m], in_=wqbT[k, m, :, :])
                    wkb_t = [[p2w.tile([128, 128], BF16, name=f"wkb{k}{m}")
                              for m in range(2)] for k in range(4)]
                    for k in range(4):
                        for m in range(2):
                            nc.sync.dma_start(out=wkb_t[k][m], in_=wkbT[k, m, :, :])
                    wvb_t = [p2w.tile([128, HPC * VD], BF16, name=f"wvb{k}")
                             for k in range(4)]
                    for k in range(4):
                        nc.sync.dma_start(out=wvb_t[k], in_=wvbT[k, :, :])
                    if flags['bvb']:
                        bvb_t = p2w.tile([1, HPC * VD], F32R)
                        nc.sync.dma_start(out=bvb_t, in_=bvb[:, :].bitcast(F32R))
                    if flags['bqb']:
                        bq_t = [p2w.tile([128, 1], F32, name=f"bqt{m}") for m in range(4)]
                        for m in range(4):
                            nc.sync.dma_start(out=bq_t[m],
                                              in_=bqb[m * 128:(m + 1) * 128]
                                              .rearrange("(a b) -> a b", b=1))
                    if flags['bkb']:
                        bk_t = [p2w.tile([128, 1], F32, name=f"bkt{m}") for m in range(2)]
                        for m in range(2):
                            nc.sync.dma_start(out=bk_t[m],
                                              in_=bkb[m * 128:(m + 1) * 128]
                                              .rearrange("(a b) -> a b", b=1))

                    # -- pass B: k/v up-proj over all blocks (AG-kv lands first) --
                    for s in range(NBLK):
                        tsl = slice(s * R, (s + 1) * R)
                        kn_t = [p2lat.tile([128, R], BF16, tag=f"kn{k}", name=f"kn{k}")
                                for k in range(4)]
                        for k in range(4):
                            nc.sync.dma_start(out=kn_t[k],
                                              in_=agk_out[s, k * 128:(k + 1) * 128, :])
                        # k_nope (weights pre-scaled x WS)
                        for m in range(2):
                            ps = p2ps.tile([128, R], F32, tag="p2acc")
                            for k in range(4):
                                nc.tensor.matmul(ps[:, :], wkb_t[k][m][:, :], kn_t[k][:, :],
                                                 start=(k == 0), stop=(k == 3))
                            if flags['bkb']:
                                nc.vector.tensor_scalar_add(kdr[m][:, 0, tsl], ps[:, :],
                                                            bk_t[m][:, :])
                            else:
                                nc.vector.tensor_copy(out=kdr[m][:, 0, tsl], in_=ps[:, :])
                        # v (token-major, unscaled)
                        for mt in range(4):
                            ps = p2ps.tile([128, HPC * VD], F32, tag="p2v")
                            if flags['bvb']:
                                nc.tensor.matmul(ps[:, :], ones_row[:, :], bvb_t[:, :],
                                                 start=True, stop=False)
                            for k in range(4):
                                nc.tensor.matmul(
                                    ps[:, :],
                                    kn_t[k][:, mt * 128:(mt + 1) * 128],
                                    wvb_t[k][:, :],
                                    start=(k == 0 and not flags['bvb']),
                                    stop=(k == 3))
                            nc.scalar.activation(out=vt[:, s * 4 + mt, :], in_=ps[:, :],
                                                 func=AF.Copy)
                    # k_pe tail (depends on the late AG-rope piece; kept out of
                    # the main loop so its DMAs don't head-of-line-block kn loads)
                    for s in range(NBLK):
                        tsl = slice(s * R, (s + 1) * R)
                        kp = p2tmp.tile([64, R], BF16, tag="kp")
                        nc.sync.dma_start(out=kp, in_=agr_out[s, :, :])
                        for i in range(HPC):
                            nc.vector.tensor_copy(out=kdr[i][0:64, 1, tsl],
                                                  in_=kp[:, :])
                    # -- pass A: q up-proj over all blocks (needs AG-q) --
                    for s in range(NBLK):
                        tsl = slice(s * R, (s + 1) * R)
                        pos = (s % (S // R)) * R       # position within batch
                        psl = slice(pos, pos + R)
                        qn_t = [p2lat.tile([128, R], BF16, tag=f"qn{k}", name=f"qn{k}")
                                for k in range(4)]
                        for k in range(4):
                            nc.sync.dma_start(out=qn_t[k],
                                              in_=agq_out[s, k * 128:(k + 1) * 128, :])
                        # q_b (weights pre-scaled x WS -> psum x WS); rope uses a
                        # second row-swapped matmul so the rotation is 3 aligned
                        # DVE ops with no cross-partition DMA shuffles:
                        #   q_rope = ps_rope * [c;c;c;c] + ps_swap * [-s;s;-s;s]
                        ps_rope = None
                        for m in range(4):
                            ps = p2ps.tile([128, R], F32, tag=f"p2q{m}",
                                           name=f"p2q{s}_{m}", bufs=1)
                            for k in range(4):
                                nc.tensor.matmul(ps[:, :], wqb_t[k][m][:, :], qn_t[k][:, :],
                                                 start=(k == 0), stop=(k == 3))
                            if m < 2:
                                if flags['bqb']:
                                    nc.vector.tensor_scalar_add(qdr[m][:, 0, tsl],
                                                                ps[:, :], bq_t[m][:, :])
                                else:
                                    nc.vector.tensor_copy(out=qdr[m][:, 0, tsl],
                                                          in_=ps[:, :])
                            elif m == 2:
                                ps_rope = ps
                            else:
                                xa = p2tmp.tile([128, R], F32, tag="xa")
                                xb = p2tmp.tile([128, R], F32, tag="xb")
                                if flags['bqb']:
                                    ra = p2tmp.tile([128, R], F32, tag="ra")
                                    nc.vector.tensor_scalar_add(ra[:, :], ps_rope[:, :],
                                                                bq_t[2][:, :])
                                    rb_ = p2tmp.tile([128, R], F32, tag="rb")
                                    nc.vector.tensor_scalar_add(rb_[:, :], ps[:, :],
                                                                bq_t[3][:, :])
                                    nc.vector.tensor_mul(xa[:, :], ra[:, :], tq1[:, psl])
                                    nc.vector.tensor_mul(xb[:, :], rb_[:, :], tq2[:, psl])
                                else:
                                    nc.vector.tensor_mul(xa[:, :], ps_rope[:, :],
                                                         tq1[:, psl])
                                    nc.vector.tensor_mul(xb[:, :], ps[:, :],
                                                         tq2[:, psl])
                                qr = p2tmp.tile([128, R], F8, tag="qr")
                                nc.vector.tensor_add(qr[:, :], xa[:, :], xb[:, :])
                                nc.sync.dma_start(out=qdr[0][0:64, 1, tsl],
                                                  in_=qr[0:64, :])
                                nc.sync.dma_start(out=qdr[1][0:64, 1, tsl],
                                                  in_=qr[64:128, :])

                # ================= PHASE 3: attention =============================
                with tc.tile_pool(name=f"p3m{_rep}", bufs=1) as p3m, \
                     tc.tile_pool(name=f"p3p{_rep}", bufs=4) as p3p, \
                     tc.tile_pool(name=f"p3o{_rep}", bufs=3) as p3o, \
                     tc.tile_pool(name=f"p3rsa{_rep}", bufs=2) as p3rsa, \
                     tc.tile_pool(name=f"p4w{_rep}", bufs=1) as p4w:

                    mtiles = [p3m.tile([128, 512], BF16, name=f"mt{i}")
                              for i in range(nmask)]
                    for i in range(nmask):
                        nc.sync.dma_start(out=mtiles[i], in_=maskblk[i, :, :])
                    # preload wo during attention (16 heads x 4 col blocks)
                    wo_t = [[p4w.tile([128, 512], BF16, name=f"wo{k}_{n_}")
                             for n_ in range(4)] for k in range(16)]
                    for k in range(16):
                        for n_ in range(4):
                            nc.sync.dma_start(out=wo_t[k][n_], in_=woT[k, n_, :, :])

                    p3psum = tc.alloc_tile_pool(name=f"p3sc{_rep}", bufs=3,
                                                space="PSUM")
                    p3out = tc.alloc_tile_pool(name=f"p3out{_rep}", bufs=3,
                                               space="PSUM")
                    p3rep = tc.alloc_tile_pool(name=f"p3rep{_rep}", bufs=2,
                                               space="PSUM")
                    p3sc = p3psum
                    ESCALE = SCALE / (WS * WS)
                    chunk = 0
                    for lh in range(HPC):
                        for b in range(B):
                            for qc in range(S // 512):
                                qsl = slice(b * S + qc * 512, b * S + (qc + 1) * 512)
                                out_ps = p3out.tile([128, 512], F32, tag="outp")
                                rs_acc = p3rsa.tile([128, 512], BF16, tag="rsacc")
                                # denominator accumulation mostly on gpsimd
                                # (~1.3us/op there vs ~0.7us on DVE, but DVE
                                # also carries mask/recip/ao) - 3:1 split
                                # balances the two engines
                                rs_eng = nc.vector if chunk % 4 == 3 else nc.gpsimd
                                chunk += 1
                                kbs = [kb for kb in range(S // 128)
                                       if cls[qc][kb] != SKIP]
                                for i, kb in enumerate(kbs):
                                    ksl = slice(b * S + kb * 128, b * S + kb * 128 + 128)
                                    sc = p3sc.tile([128, 512], F32, tag="sc")
                                    nc.tensor.matmul(sc[:, :], kdr[lh][:, :, ksl],
                                                     qdr[lh][:, :, qsl],
                                                     start=True, stop=True,
                                                     perf_mode=PM.DoubleRow)
                                    P = p3p.tile([128, 512], BF16, tag="P")
                                    nc.scalar.activation(out=P[:, :], in_=sc[:, :],
                                                         func=AF.Exp, scale=ESCALE)
                                    if cls[qc][kb] >= 0:
                                        nc.vector.tensor_mul(P[:, :], P[:, :],
                                                             mtiles[cls[qc][kb]][:, :])
                                    last = (i == len(kbs) - 1)
                                    nc.tensor.matmul(
                                        out_ps[:, :],
                                        vt[:, b * 16 + kb, lh * VD:(lh + 1) * VD],
                                        P[:, :], start=(i == 0), stop=last,
                                        skip_group_check=True)
                                    if i == 0:
                                        rs_eng.tensor_copy(out=rs_acc[:, :],
                                                           in_=P[:, :])
                                    else:
                                        rs_eng.tensor_add(rs_acc[:, :],
                                                          rs_acc[:, :], P[:, :])
                                # fused rowsum + broadcast: all-ones lhsT makes
                                # every output row the column sum of rs_acc; the
                                # [128,512] reciprocal costs the same as [1,512]
                                # (free-size-bound), so this replaces the [1,512]
                                # rowsum matmul, F32R cast, rep matmul and its
                                # PSUM drain
                                rep = p3rep.tile([128, 512], F32, tag="rep")
                                nc.tensor.matmul(rep[:, :], ones_sq[:, :],
                                                 rs_acc[:, :], start=True, stop=True)
                                inv_sb = p3o.tile([128, 512], F32, tag="invsb")
                                nc.vector.reciprocal(out=inv_sb[:, :], in_=rep[:, :])
                                ao = p3o.tile([128, 512], BF16, tag="ao")
                                nc.vector.tensor_mul(ao[:, :], out_ps[:, :], inv_sb[:, :])
                                nc.sync.dma_start(
                                    out=a2a_in[lh][b * 4 + qc, :, :],
                                    in_=ao[:, :])
                        nc.gpsimd.collective_compute(
                            "AllToAll", ALU.bypass,
                            replica_groups=[list(range(NC))],
                            ins=[a2a_in[lh].opt()], outs=[a2a_out[lh].opt()])

                    p3rep.release()
                    p3out.release()
                    p3psum.release()

                    # ============= PHASE 4: output projection =====================
                    with tc.tile_pool(name=f"p4l{_rep}", bufs=1) as p4l, \
                         tc.tile_pool(name=f"p4o{_rep}", bufs=3) as p4o, \
                         tc.tile_pool(name=f"p4ps{_rep}", bufs=4, space="PSUM") as p4ps:

                        lt = [p4l.tile([128, 512], BF16, name=f"lt{k}") for k in range(16)]
                        for k in range(16):
                            nc.sync.dma_start(out=lt[k], in_=a2a_out[k % 2][k // 2, :, :])
                        if flags['wob']:
                            wob_t = p4l.tile([1, DIM], F32R)
                            nc.sync.dma_start(out=wob_t, in_=wob[:, :].bitcast(F32R))
                        # contraction order: even heads (from a2a#0) first so the
                        # PE can start before a2a#1 lands
                        korder = [k for k in range(16) if k % 2 == 0] + \
                                 [k for k in range(16) if k % 2 == 1]
                        for n_ in range(4):
                            for m in range(4):
                                ps = p4ps.tile([128, 512], F32, tag="p4acc")
                                if flags['wob']:
                                    nc.tensor.matmul(ps[:, :], ones_row[:, :],
                                                     wob_t[:, n_ * 512:(n_ + 1) * 512],
                                                     start=True, stop=False)
                                for j, k in enumerate(korder):
                                    nc.tensor.matmul(ps[:, :],
                                                     lt[k][:, m * 128:(m + 1) * 128],
                                                     wo_t[k][n_][:, :],
                                                     start=(j == 0 and not flags['wob']),
                                                     stop=(j == 15))
                                ob = p4o.tile([128, 512], F32, tag="ob")
                                nc.scalar.activation(out=ob[:, :], in_=ps[:, :],
                                                     func=AF.Copy)
                                nc.sync.dma_start(
                                    out=out_c[m * 128:(m + 1) * 128,
                                              n_ * 512:(n_ + 1) * 512],
                                    in_=ob[:, :])

                qkv.release()

    nc.finalize()
    return nc


_ROPE_PERM = np.concatenate([np.arange(0, ROPE, 2), np.arange(1, ROPE, 2)])

_CACHE = {}


def _prep_inputs(inputs):
    """Host-side slicing/permutation -> (schedule key data, per-core in_maps)."""
    x = np.ascontiguousarray(np.asarray(inputs['x'], np.float32).reshape(N, DIM))
    mask = np.asarray(inputs['mask'])
    cls, blocks = _classify_mask(mask)

    cos_t, sin_t = _rope_tables()            # [32, S]
    # q-rope combine tables: q_rope = ps_rope*trigA + ps_swap*trigB
    trigq1 = np.concatenate([cos_t, cos_t, cos_t, cos_t], 0)   # [128, S]
    trigq2 = np.concatenate([-sin_t, sin_t, -sin_t, sin_t], 0)
    sgn = np.concatenate([-np.ones(32), np.ones(32), -np.ones(32), np.ones(32)]
                         ).astype(np.float32)[:, None]

    wq_a = np.asarray(inputs['wq_a_w'], np.float32)            # [QLR, DIM]
    # k_pe rows of wkv_a carry the q/k WS pre-scale (k side); latent rows don't.
    wkv_a = np.asarray(inputs['wkv_a_w'], np.float32)          # [KVLR+ROPE, DIM]
    wkv_a_p = np.concatenate([wkv_a[:KVLR], wkv_a[KVLR:][_ROPE_PERM] * WS], 0)
    bkva = np.asarray(inputs['wkv_a_b'], np.float32)
    bkva_p = np.concatenate([bkva[:KVLR], bkva[KVLR:][_ROPE_PERM] * WS], 0)

    # q/k up-projections pre-scaled by WS (compensated in the exp scale);
    # v rows stay unscaled.
    wq_b = np.asarray(inputs['wq_b_w'], np.float32).reshape(H, QKD, QLR) * WS
    bq_b = np.asarray(inputs['wq_b_b'], np.float32).reshape(H, QKD) * WS
    wkv_b = np.asarray(inputs['wkv_b_w'], np.float32).reshape(H, NOPE + VD, KVLR).copy()
    bkv_b = np.asarray(inputs['wkv_b_b'], np.float32).reshape(H, NOPE + VD).copy()
    wkv_b[:, :NOPE] *= WS
    bkv_b[:, :NOPE] *= WS
    wo = np.asarray(inputs['wo_w'], np.float32)                # [DIM, H*VD]

    def blk(wT, nk, nm, mcols):
        return np.ascontiguousarray(
            wT.reshape(nk, 128, nm, mcols).transpose(0, 2, 1, 3)).astype(NPBF)

    wkva_pad = np.concatenate(
        [wkv_a_p, np.zeros((64, DIM), np.float32)], 0)

    def mwide(wblk):  # [nk, nm, 128, mc] -> [nk, 128, nm*mc] (one DMA per k)
        nk, nm, p, mc = wblk.shape
        return np.ascontiguousarray(
            wblk.transpose(0, 2, 1, 3).reshape(nk, p, nm * mc))

    shared = {
        'wqaT': mwide(blk(np.ascontiguousarray(wq_a.T), 16, 4, 128)),
        'bqa': np.asarray(inputs['wq_a_b'], np.float32),
        'wkvaT': mwide(blk(np.ascontiguousarray(wkva_pad.T), 16, 5, 128)),
        'bkva': bkva_p,
        'qnw': np.asarray(inputs['q_norm_w'], np.float32),
        'kvnw': np.asarray(inputs['kv_norm_w'], np.float32),
        'trigq1': np.ascontiguousarray(trigq1),
        'trigq2': np.ascontiguousarray(trigq2),
        'sgn': sgn,
        'woT': blk(np.ascontiguousarray(wo.T), 16, 4, 512),
        'wob': np.asarray(inputs['wo_b'], np.float32)[None, :],
        'maskblk': blocks if len(blocks) else np.zeros((1, 128, 512), NPBF),
    }

    # rope rows: [h0 evens(32), h0 odds(32), h1 evens, h1 odds]; the swap chunk
    # exchanges each even/odd 32-row group pair
    _SW = np.concatenate([np.arange(32, 64), np.arange(0, 32),
                          np.arange(96, 128), np.arange(64, 96)])
    in_maps = []
    for c in range(NC):
        h0, h1 = 2 * c, 2 * c + 1
        rope_rows = np.concatenate([wq_b[h0, NOPE:][_ROPE_PERM],
                                    wq_b[h1, NOPE:][_ROPE_PERM]], 0)
        brope = np.concatenate([bq_b[h0, NOPE:][_ROPE_PERM],
                                bq_b[h1, NOPE:][_ROPE_PERM]], 0)
        wqb_c = np.concatenate([
            wq_b[h0, :NOPE], wq_b[h1, :NOPE], rope_rows, rope_rows[_SW]], 0)
        bqb_c = np.concatenate([
            bq_b[h0, :NOPE], bq_b[h1, :NOPE], brope, brope[_SW]], 0)
        wkb_c = np.concatenate([wkv_b[h0, :NOPE], wkv_b[h1, :NOPE]], 0)
        bkb_c = np.concatenate([bkv_b[h0, :NOPE], bkv_b[h1, :NOPE]], 0)
        wvb_c = np.concatenate([wkv_b[h0, NOPE:], wkv_b[h1, NOPE:]], 0)
        bvb_c = np.concatenate([bkv_b[h0, NOPE:], bkv_b[h1, NOPE:]], 0)
        pos = (c % (S // R)) * R
        cos_c, sin_c = cos_t[:, pos:pos + R], sin_t[:, pos:pos + R]
        m = dict(shared)
        m.update({
            'x_c': np.ascontiguousarray(x[c * R:(c + 1) * R].T)
                   .reshape(DIM // 128, 128, R).astype(NPBF),
            'wqbT': blk(np.ascontiguousarray(wqb_c.T), 4, 4, 128),
            'bqb': bqb_c,
            'wkbT': blk(np.ascontiguousarray(wkb_c.T), 4, 2, 128),
            'bkb': bkb_c,
            'wvbT': np.ascontiguousarray(wvb_c.T).reshape(4, 128, HPC * VD).astype(NPBF),
            'bvb': bvb_c[None, :],
            'trig1c': np.ascontiguousarray(np.concatenate([cos_c, sin_c], 0)),
            'trig2c': np.ascontiguousarray(np.concatenate([sin_c, cos_c], 0)),
        })
        in_maps.append(m)
    return cls, in_maps


class _Runner:
    """Compile once, execute many times on the 8 axon-tunneled NeuronCores."""

    def __init__(self, nc):
        import jax
        from jax.experimental.shard_map import shard_map
        from jax.sharding import Mesh, PartitionSpec
        from concourse import bass2jax, mybir as _mybir
        bass2jax.install_neuronx_cc_hook()
        self.jax = jax
        in_names, out_names, out_avals, zero_outs = [], [], [], []
        partition_name = (nc.partition_id_tensor.name
                          if nc.partition_id_tensor else None)
        for alloc in nc.m.functions[0].allocations:
            if not isinstance(alloc, _mybir.MemoryLocationSet):
                continue
            name = alloc.memorylocations[0].name
            if alloc.kind == "ExternalInput":
                if name != partition_name:
                    in_names.append(name)
            elif alloc.kind == "ExternalOutput":
                shape = tuple(alloc.tensor_shape)
                dtype = _mybir.dt.np(alloc.dtype)
                out_names.append(name)
                out_avals.append(jax.core.ShapedArray(shape, dtype))
                zero_outs.append(np.zeros(shape, dtype))
        self.n_params = len(in_names)
        self.in_names = list(in_names)
        self.out_names = out_names
        self.out_avals = out_avals
        self.zero_outs = zero_outs
        all_in = in_names + out_names
        if partition_name is not None:
            all_in.append(partition_name)

        def _body(*args):
            operands = list(args)
            if partition_name is not None:
                operands.append(bass2jax.partition_id_tensor())
            outs = bass2jax._bass_exec_p.bind(
                *operands,
                out_avals=tuple(out_avals),
                in_names=tuple(all_in),
                out_names=tuple(out_names),
                lowering_input_output_aliases=(),
                sim_require_finite=True,
                sim_require_nnan=True,
                nc=nc)
            return tuple(outs)

        devices = jax.devices()[:NC]
        self.mesh = Mesh(np.asarray(devices), ("core",))
        n_out = len(out_names)
        in_specs = (PartitionSpec("core"),) * (self.n_params + n_out)
        out_specs = (PartitionSpec("core"),) * n_out
        donate = tuple(range(self.n_params, self.n_params + n_out))
        self.fn = jax.jit(
            shard_map(_body, mesh=self.mesh, in_specs=in_specs,
                      out_specs=out_specs, check_rep=False),
            donate_argnums=donate, keep_unused=True)

    def concat_inputs(self, in_maps):
        return [np.concatenate([np.asarray(in_maps[c][nm])
                                for c in range(NC)], axis=0)
                for nm in self.in_names]

    def zeros(self):
        return [np.zeros((NC * z.shape[0], *z.shape[1:]), z.dtype)
                for z in self.zero_outs]

    def __call__(self, concat_in, concat_zeros):
        out = self.fn(*concat_in, *concat_zeros)
        return out

    def run(self, in_maps):
        outs = self(self.concat_inputs(in_maps), self.zeros())
        res = []
        for c in range(NC):
            res.append({nm: np.asarray(outs[i]).reshape(NC, *self.out_avals[i].shape)[c]
                        for i, nm in enumerate(self.out_names)})
        return res


def _get_exec(cls, nmask, flags):
    key = (tuple(tuple(r) for r in cls), nmask,
           tuple(flags['ba']), flags['bqb'], flags['bkb'], flags['bvb'],
           flags['wob'])
    if key not in _CACHE:
        nc = _build(cls, nmask, flags)
        _CACHE[key] = _Runner(nc)
    return _CACHE[key]


def kernel(**inputs):
    cls, in_maps = _prep_inputs(inputs)
    nmask = max(len(in_maps[0]['maskblk']), 1)
    flags = {
        'ba': (bool(np.any(inputs['wq_a_b'])), bool(np.any(inputs['wkv_a_b']))),
        'bqb': bool(np.any(inputs['wq_b_b'])),
        'bkb': bool(np.any(np.asarray(inputs['wkv_b_b']).reshape(H, NOPE + VD)[:, :NOPE])),
        'bvb': bool(np.any(np.asarray(inputs['wkv_b_b']).reshape(H, NOPE + VD)[:, NOPE:])),
        'wob': bool(np.any(inputs['wo_b'])),
    }
    runner = _get_exec(cls, nmask, flags)
    results = runner.run(in_maps)
    out = np.concatenate([results[c]["out"] for c in range(NC)], 0)
    return out.reshape(B, S, DIM)
